# revision 7
# baseline (speedup 1.0000x reference)
"""Conv4d (kernel 3^4, circular, grouped-over-time) on 8 TRN2 NeuronCores.

Math: res[b,co,t] = sum_g conv3d_valid(pad_wrap1(x[b,:,s=t-1+g]), W[g]) + bias,
with s circular over the 16 time slices.

Device scheme (per core = one (batch, 8-time-slice) shard):
  - outputs processed in pairs (t, t+1); PSUM partitions = (t-sel u, c_out)
  - contraction K = (input-slice-sel j, c_in) over pair-tiles of two
    consecutive padded slices stacked on partitions
  - per (kd,kh,kw) tap and output pair: 2 matmuls, K=128 M=128 N=512 bf16:
      L-block: slices (t-1, t),  g = j - u      (g=-1 entry zeroed)
      H-block: slices (t+1, t+2), g = j - u + 2 (g=3 entry zeroed)
  - x staged in SBUF as THREE kw-pre-shifted 16-wide cube copies so every
    rhs window is a contiguous 32B-aligned 2-level AP (a 2-byte-misaligned
    bf16 moving window costs ~35ns/matmul on the PE)
  - loop order tap-outer / chunk-inner: 8 consecutive matmuls share one
    lhsT slice, minimizing unhidden LDWEIGHTS time; taps ordered kw-major
    (0, 2, 1) to relax DMA deadlines of the shifted copies
  - PSUM evacuation split across DVE (tensor_scalar_add) and Act
    (activation Identity + bias) into a bf16 stage; host upcasts to f32
"""
import numpy as np

B, C, S, KW = 4, 64, 16, 3
SP = S + 2            # padded d/h extent
CUBE = SP * SP * S    # 5184: one 16-wide shifted cube copy per channel
NCORES = 8
TSH = S * B // NCORES  # 8 output time slices per core
KWORD = (0, 2, 1)      # kw processing order (shifted-copy DMA slack)

_PROGRAM = None


def _build_program():
    import concourse.bacc as bacc
    import concourse.mybir as mybir
    import concourse.tile as tile

    nc = bacc.Bacc("TRN2", target_bir_lowering=False, debug=False,
                   num_devices=NCORES)
    bf16 = mybir.dt.bfloat16
    f32 = mybir.dt.float32
    IDENT = mybir.ActivationFunctionType.Identity

    xs_d = [nc.dram_tensor(f"xs{kw}", [5, 128, CUBE], bf16,
                           kind="ExternalInput").ap() for kw in range(KW)]
    wl_d = nc.dram_tensor("wl", [128, 27 * 128], bf16, kind="ExternalInput").ap()
    wh_d = nc.dram_tensor("wh", [128, 27 * 128], bf16, kind="ExternalInput").ap()
    bias_d = nc.dram_tensor("bias2", [128, 1], f32, kind="ExternalInput").ap()
    y_d = nc.dram_tensor("y", [TSH, C, S * S * S], bf16,
                         kind="ExternalOutput").ap()

    with tile.TileContext(nc) as tc:
        with (
            tc.tile_pool(name="xp", bufs=1) as xpool,
            tc.tile_pool(name="wp", bufs=1) as wpool,
            tc.tile_pool(name="st", bufs=2) as spool,
            tc.tile_pool(name="ps", bufs=8, space="PSUM") as pspool,
        ):
            wlt = wpool.tile([128, 27 * 128], bf16)
            wht = wpool.tile([128, 27 * 128], bf16)
            bias_t = wpool.tile([128, 1], f32)
            xts = [[xpool.tile([128, CUBE], bf16, name=f"xt{kw}_{k}")
                    for k in range(5)] for kw in range(KW)]

            quarter = 5 * SP * S  # ~4.5 d-planes per piece (18 planes = 4 pieces)

            def xdma(q, kw, k, p, np_=1):
                lo = p * quarter
                hi = min((p + np_) * quarter, CUBE)
                q.dma_start(xts[kw][k][:, lo:hi], xs_d[kw][k][:, lo:hi])

            wpiece = 7 * 128

            def wdma(q, t, p):
                lo, hi = p * wpiece, min((p + 1) * wpiece, 27 * 128)
                q.dma_start(t[:, lo:hi], (wl_d if t is wlt else wh_d)[:, lo:hi])

            # Early: first matmuls (kw=0, tap 0, chunks 0-7) touch all of
            # C0-cube0 plus the first wl piece.  C0-cube0 split 4 ways over
            # two queues; C2-cube0 on scalar (needed ~26us), C1 after it
            # (needed ~45us); weights lead on gpsimd.
            wdma(nc.gpsimd, wlt, 0)
            xdma(nc.sync, 0, 0, 0)
            xdma(nc.gpsimd, 0, 0, 1)
            xdma(nc.sync, 0, 0, 2)
            xdma(nc.gpsimd, 0, 0, 3)
            xdma(nc.scalar, 2, 0, 0, 2)
            xdma(nc.scalar, 2, 0, 2, 2)
            nc.gpsimd.dma_start(bias_t[:], bias_d)
            for p in (1, 2, 3):
                wdma(nc.gpsimd, wlt, p)
            xdma(nc.sync, 0, 1, 0, 2)
            xdma(nc.sync, 0, 1, 2, 2)
            xdma(nc.scalar, 1, 0, 0, 2)
            xdma(nc.scalar, 1, 0, 2, 2)
            for p in range(4):
                wdma(nc.gpsimd, wht, p)
            for k in (1, 2, 3, 4):
                xdma(nc.gpsimd, 2, k, 0, 2)
                xdma(nc.gpsimd, 2, k, 2, 2)
            xdma(nc.scalar, 1, 1, 0, 2)
            xdma(nc.scalar, 1, 1, 2, 2)
            for k in (2, 3, 4):
                xdma(nc.sync, 0, k, 0, 2)
                xdma(nc.sync, 0, k, 2, 2)
                xdma(nc.scalar, 1, k, 0, 2)
                xdma(nc.scalar, 1, k, 2, 2)

            # [p, d(18), hw(18 h-rows of 16)]
            xvs = [[xt.rearrange("p (d hw) -> p d hw", d=SP, hw=SP * S)
                    for xt in row] for row in xts]

            for u in range(TSH // 2):  # output pair
                banks = [pspool.tile([128, 512], f32, name="bank")
                         for _ in range(8)]
                stage = spool.tile([128, S * S * S], bf16, name="stage")
                slot = 0
                for blk in range(2):   # L (wlt) then H (wht)
                    wt = wlt if blk == 0 else wht
                    for kwi, kw in enumerate(KWORD):
                        xv = xvs[kw][u + blk]
                        for kd in range(KW):
                            for kh in range(KW):
                                i = kwi * 9 + kd * KW + kh
                                lhsT = wt[:, i * 128:(i + 1) * 128]
                                for c in range(8):
                                    rhs = xv[:, 2 * c + kd:2 * c + kd + 2,
                                             kh * S:(kh + S) * S]
                                    nc.tensor.matmul(
                                        banks[c][:], lhsT, rhs,
                                        start=(slot == 0), stop=(slot == 53),
                                    )
                                slot += 1
                qy = nc.sync if u % 2 == 0 else nc.gpsimd
                for c in range(8):
                    sl = stage[:, c * 512:(c + 1) * 512]
                    if c % 2 == 0:
                        nc.vector.tensor_scalar_add(sl, banks[c][:], bias_t[:])
                    else:
                        nc.scalar.activation(sl, banks[c][:], IDENT,
                                             bias=bias_t[:])
                    qy.dma_start(y_d[2 * u][:, c * 512:(c + 1) * 512],
                                 stage[0:C, c * 512:(c + 1) * 512])
                    qy.dma_start(y_d[2 * u + 1][:, c * 512:(c + 1) * 512],
                                 stage[C:128, c * 512:(c + 1) * 512])

    nc.compile()
    return nc


def _host_prep(x, weight, bias):
    """Build per-core input maps (bf16 activations/weights, f32 bias)."""
    import ml_dtypes

    xpad = np.pad(x, ((0, 0), (0, 0), (0, 0), (1, 1), (1, 1), (1, 1)),
                  mode="wrap").astype(np.float32)  # (B, C, S, 18,18,18)

    # kw-pre-shifted 16-wide copies: xsh[kw][b,ci,s,d,h,w16] = xpad[..., w16+kw]
    xsh = [np.ascontiguousarray(xpad[..., kw:kw + S]).astype(ml_dtypes.bfloat16)
           for kw in range(KW)]

    # weight block-banded lhsT tiles: [128=(j,ci), 27*128=(tap,(u,co))]
    # tap index i = kwi*9 + kd*3 + kh with kw = KWORD[kwi]
    wl = np.zeros((128, 27, 128), dtype=np.float32)
    wh = np.zeros((128, 27, 128), dtype=np.float32)
    for kwi, kw in enumerate(KWORD):
        for kd in range(KW):
            for kh in range(KW):
                i = kwi * 9 + kd * KW + kh
                for j in range(2):
                    for u in range(2):
                        gl = j - u
                        if 0 <= gl < KW:
                            wl[j * C:(j + 1) * C, i, u * C:(u + 1) * C] = \
                                weight[gl, :, :, kd, kh, kw].T
                        gh = j - u + 2
                        if 0 <= gh < KW:
                            wh[j * C:(j + 1) * C, i, u * C:(u + 1) * C] = \
                                weight[gh, :, :, kd, kh, kw].T
    wl = wl.reshape(128, 27 * 128).astype(ml_dtypes.bfloat16)
    wh = wh.reshape(128, 27 * 128).astype(ml_dtypes.bfloat16)
    bias2 = np.concatenate([bias, bias]).astype(np.float32).reshape(128, 1)

    in_maps = []
    for core in range(NCORES):
        b = core // 2
        t0 = TSH * (core % 2)
        m = {"wl": wl, "wh": wh, "bias2": bias2}
        for kw in range(KW):
            xs = np.empty((5, 128, CUBE), dtype=ml_dtypes.bfloat16)
            for k in range(5):
                sa = (t0 - 1 + 2 * k) % S
                sb = (t0 + 2 * k) % S
                xs[k, 0:C] = xsh[kw][b, :, sa].reshape(C, CUBE)
                xs[k, C:128] = xsh[kw][b, :, sb].reshape(C, CUBE)
            m[f"xs{kw}"] = xs
        in_maps.append(m)
    return in_maps


LAST_RESULTS = None


def kernel(x, weight, bias, _trace=False):
    global _PROGRAM, LAST_RESULTS
    from concourse import bass_utils

    x = np.asarray(x, dtype=np.float32)
    weight = np.asarray(weight, dtype=np.float32)
    bias = np.asarray(bias, dtype=np.float32)

    if _PROGRAM is None:
        _PROGRAM = _build_program()
    nc = _PROGRAM

    in_maps = _host_prep(x, weight, bias)
    res = bass_utils.run_bass_kernel_spmd(
        nc, in_maps, core_ids=list(range(NCORES)), trace=_trace
    )
    LAST_RESULTS = res

    out = np.empty((B, C, S, S, S, S), dtype=np.float32)
    for core in range(NCORES):
        b = core // 2
        t0 = TSH * (core % 2)
        y = np.asarray(res.results[core]["y"], dtype=np.float32)  # (TSH, C, 4096)
        out[b, :, t0:t0 + TSH] = y.transpose(1, 0, 2).reshape(C, TSH, S, S, S)
    return out


# revision 9
# speedup vs baseline: 1.3928x; 1.3928x over previous
"""Conv4d (kernel 3^4, circular, grouped-over-time) on 8 TRN2 NeuronCores.

Math: res[b,co,t] = sum_g conv3d_valid(pad_wrap1(x[b,:,s=t-1+g]), W[g]) + bias,
with s circular over the 16 time slices.

Device scheme (per core = one (batch, 8-time-slice) shard):
  - outputs processed in pairs (t, t+1); PSUM partitions = (t-sel u, c_out)
  - contraction K = (input-slice-sel j, c_in) over pair-tiles of two
    consecutive padded slices stacked on partitions
  - per (kd,kh,kw) tap and output pair: 2 matmuls, K=128 M=128 N=512 bf16:
      L-block: slices (t-1, t),  g = j - u      (g=-1 entry zeroed)
      H-block: slices (t+1, t+2), g = j - u + 2 (g=3 entry zeroed)
  - x staged in SBUF as THREE kw-pre-shifted 16-wide cube copies so every
    rhs window is a contiguous 32B-aligned 2-level AP (a 2-byte-misaligned
    bf16 moving window costs ~35ns/matmul on the PE)
  - loop order tap-outer / chunk-inner: 8 consecutive matmuls share one
    lhsT slice, minimizing unhidden LDWEIGHTS time; taps ordered kw-major
    (0, 2, 1) to relax DMA deadlines of the shifted copies
  - PSUM evacuation split across DVE (tensor_scalar_add) and Act
    (activation Identity + bias) into a bf16 stage; host upcasts to f32
"""
import numpy as np

B, C, S, KW = 4, 64, 16, 3
SP = S + 2            # padded d/h extent
CUBE = SP * SP * S    # 5184: one 16-wide shifted cube copy per channel
NCORES = 8
TSH = S * B // NCORES  # 8 output time slices per core
KWORD = (0, 2, 1)      # kw processing order (shifted-copy DMA slack)

_PROGRAM = None


def _build_program():
    import concourse.bacc as bacc
    import concourse.mybir as mybir
    import concourse.tile as tile

    nc = bacc.Bacc("TRN2", target_bir_lowering=False, debug=False,
                   num_devices=NCORES)
    bf16 = mybir.dt.bfloat16
    f32 = mybir.dt.float32
    IDENT = mybir.ActivationFunctionType.Identity

    xs_d = [nc.dram_tensor(f"xs{kw}", [5, 128, CUBE], bf16,
                           kind="ExternalInput").ap() for kw in range(KW)]
    wl_d = nc.dram_tensor("wl", [128, 27 * 128], bf16, kind="ExternalInput").ap()
    wh_d = nc.dram_tensor("wh", [128, 27 * 128], bf16, kind="ExternalInput").ap()
    bias_d = nc.dram_tensor("bias2", [128, 1], f32, kind="ExternalInput").ap()
    y_d = nc.dram_tensor("y", [TSH, C, S * S * S], bf16,
                         kind="ExternalOutput").ap()

    with tile.TileContext(nc) as tc:
        with (
            tc.tile_pool(name="xp", bufs=1) as xpool,
            tc.tile_pool(name="wp", bufs=1) as wpool,
            tc.tile_pool(name="st", bufs=2) as spool,
            tc.tile_pool(name="ps", bufs=8, space="PSUM") as pspool,
        ):
            wlt = wpool.tile([128, 27 * 128], bf16)
            wht = wpool.tile([128, 27 * 128], bf16)
            bias_t = wpool.tile([128, 1], f32)
            xts = [[xpool.tile([128, CUBE], bf16, name=f"xt{kw}_{k}")
                    for k in range(5)] for kw in range(KW)]

            half = 9 * SP * S  # 9 d-planes

            def xdma(q, kw, k, p):
                q.dma_start(xts[kw][k][:, p * half:(p + 1) * half],
                            xs_d[kw][k][:, p * half:(p + 1) * half])

            wpiece = 7 * 128

            def wdma(q, t, p):
                lo, hi = p * wpiece, min((p + 1) * wpiece, 27 * 128)
                q.dma_start(t[:, lo:hi], (wl_d if t is wlt else wh_d)[:, lo:hi])

            # Early: first matmuls (kw=0, tap 0, chunks 0-7) touch all of
            # C0-cube0 plus the first wl piece.  Weights lead on gpsimd;
            # C2/C1 copies follow on gpsimd/scalar (needed ~25us/~45us in).
            wdma(nc.gpsimd, wlt, 0)
            xdma(nc.sync, 0, 0, 0)
            xdma(nc.sync, 0, 0, 1)
            xdma(nc.gpsimd, 2, 0, 0)
            xdma(nc.gpsimd, 2, 0, 1)
            nc.gpsimd.dma_start(bias_t[:], bias_d)
            xdma(nc.sync, 0, 1, 0)
            xdma(nc.sync, 0, 1, 1)
            for p in (1, 2, 3):
                wdma(nc.gpsimd, wlt, p)
            xdma(nc.scalar, 1, 0, 0)
            xdma(nc.scalar, 1, 0, 1)
            for p in range(4):
                wdma(nc.gpsimd, wht, p)
            for k in (1, 2, 3, 4):
                xdma(nc.gpsimd, 2, k, 0)
                xdma(nc.gpsimd, 2, k, 1)
            xdma(nc.scalar, 1, 1, 0)
            xdma(nc.scalar, 1, 1, 1)
            for k in (2, 3, 4):
                xdma(nc.sync, 0, k, 0)
                xdma(nc.sync, 0, k, 1)
                xdma(nc.scalar, 1, k, 0)
                xdma(nc.scalar, 1, k, 1)

            # [p, d(18), hw(18 h-rows of 16)]
            xvs = [[xt.rearrange("p (d hw) -> p d hw", d=SP, hw=SP * S)
                    for xt in row] for row in xts]

            for u in range(TSH // 2):  # output pair
                banks = [pspool.tile([128, 512], f32, name="bank")
                         for _ in range(8)]
                stage = spool.tile([128, S * S * S], bf16, name="stage")
                slot = 0
                for blk in range(2):   # L (wlt) then H (wht)
                    wt = wlt if blk == 0 else wht
                    for kwi, kw in enumerate(KWORD):
                        xv = xvs[kw][u + blk]
                        for kd in range(KW):
                            for kh in range(KW):
                                i = kwi * 9 + kd * KW + kh
                                lhsT = wt[:, i * 128:(i + 1) * 128]
                                for c in range(8):
                                    rhs = xv[:, 2 * c + kd:2 * c + kd + 2,
                                             kh * S:(kh + S) * S]
                                    nc.tensor.matmul(
                                        banks[c][:], lhsT, rhs,
                                        start=(slot == 0), stop=(slot == 53),
                                    )
                                slot += 1
                for c in range(8):
                    sl = stage[:, c * 512:(c + 1) * 512]
                    if c % 2 == 0:
                        nc.vector.tensor_scalar_add(sl, banks[c][:], bias_t[:])
                    else:
                        nc.scalar.activation(sl, banks[c][:], IDENT,
                                             bias=bias_t[:])
                qy = nc.sync if u % 2 == 0 else nc.gpsimd
                qy.dma_start(y_d[2 * u], stage[0:C, :])
                qy.dma_start(y_d[2 * u + 1], stage[C:128, :])

    nc.compile()
    return nc


def _host_prep(x, weight, bias):
    """Build per-core input maps (bf16 activations/weights, f32 bias)."""
    import ml_dtypes

    xpad = np.pad(x, ((0, 0), (0, 0), (0, 0), (1, 1), (1, 1), (1, 1)),
                  mode="wrap").astype(np.float32)  # (B, C, S, 18,18,18)

    # kw-pre-shifted 16-wide copies: xsh[kw][b,ci,s,d,h,w16] = xpad[..., w16+kw]
    xsh = [np.ascontiguousarray(xpad[..., kw:kw + S]).astype(ml_dtypes.bfloat16)
           for kw in range(KW)]

    # weight block-banded lhsT tiles: [128=(j,ci), 27*128=(tap,(u,co))]
    # tap index i = kwi*9 + kd*3 + kh with kw = KWORD[kwi]
    wl = np.zeros((128, 27, 128), dtype=np.float32)
    wh = np.zeros((128, 27, 128), dtype=np.float32)
    for kwi, kw in enumerate(KWORD):
        for kd in range(KW):
            for kh in range(KW):
                i = kwi * 9 + kd * KW + kh
                for j in range(2):
                    for u in range(2):
                        gl = j - u
                        if 0 <= gl < KW:
                            wl[j * C:(j + 1) * C, i, u * C:(u + 1) * C] = \
                                weight[gl, :, :, kd, kh, kw].T
                        gh = j - u + 2
                        if 0 <= gh < KW:
                            wh[j * C:(j + 1) * C, i, u * C:(u + 1) * C] = \
                                weight[gh, :, :, kd, kh, kw].T
    wl = wl.reshape(128, 27 * 128).astype(ml_dtypes.bfloat16)
    wh = wh.reshape(128, 27 * 128).astype(ml_dtypes.bfloat16)
    bias2 = np.concatenate([bias, bias]).astype(np.float32).reshape(128, 1)

    in_maps = []
    for core in range(NCORES):
        b = core // 2
        t0 = TSH * (core % 2)
        m = {"wl": wl, "wh": wh, "bias2": bias2}
        for kw in range(KW):
            xs = np.empty((5, 128, CUBE), dtype=ml_dtypes.bfloat16)
            for k in range(5):
                sa = (t0 - 1 + 2 * k) % S
                sb = (t0 + 2 * k) % S
                xs[k, 0:C] = xsh[kw][b, :, sa].reshape(C, CUBE)
                xs[k, C:128] = xsh[kw][b, :, sb].reshape(C, CUBE)
            m[f"xs{kw}"] = xs
        in_maps.append(m)
    return in_maps


LAST_RESULTS = None


def kernel(x, weight, bias, _trace=False):
    global _PROGRAM, LAST_RESULTS
    from concourse import bass_utils

    x = np.asarray(x, dtype=np.float32)
    weight = np.asarray(weight, dtype=np.float32)
    bias = np.asarray(bias, dtype=np.float32)

    if _PROGRAM is None:
        _PROGRAM = _build_program()
    nc = _PROGRAM

    in_maps = _host_prep(x, weight, bias)
    res = bass_utils.run_bass_kernel_spmd(
        nc, in_maps, core_ids=list(range(NCORES)), trace=_trace
    )
    LAST_RESULTS = res

    out = np.empty((B, C, S, S, S, S), dtype=np.float32)
    for core in range(NCORES):
        b = core // 2
        t0 = TSH * (core % 2)
        y = np.asarray(res.results[core]["y"], dtype=np.float32)  # (TSH, C, 4096)
        out[b, :, t0:t0 + TSH] = y.transpose(1, 0, 2).reshape(C, TSH, S, S, S)
    return out


# revision 10
# speedup vs baseline: 1.3932x; 1.0003x over previous
"""Conv4d via 1D-Winograd F(2,3) along w, on 8 TRN2 NeuronCores.

Per output pair (t,t+1) the direct bf16 scheme needs 432 N=512 matmuls; the
w-axis Winograd transform replaces the 3 kw-taps by 4 pointwise products on
half the w-resolution: 288 matmuls -> 246us PE floor instead of 368us.

  input transform  (DVE/Pool, bf16):  per cube, coeffs c0..c3 over (d,h,t8):
      c0 = x[2t]-x[2t+2], c1 = x[2t+1]+x[2t+2],
      c2 = x[2t+2]-x[2t+1], c3 = x[2t+1]-x[2t+3]
  coeff GEMMs (PE): per pair, per point c, the same (j,ci)x(u,co) banded
      L/H time-block structure as the direct kernel, over taps (kd,kh);
      out m_c[(u,co), (d4,h16,t8)] accumulates 18 matmuls in PSUM
  points processed in halves {0,1} then {2,3}; m0/m1 evacuated to SBUF as
      A01 = m0+m1 (DVE) and A1b = m1+bias (Act), then with m2/m3 still in
      PSUM the inverse is
      y[2t]   = (m2 + bias) + A01          (DVE scalar_tensor_tensor)
      t23     = m2 + m3                    (DVE)
      y[2t+1] = A1b - t23                  (Pool)
"""
import numpy as np

B, C, S, KW = 4, 64, 16, 3
SP = S + 2
RCUBE = SP * SP * SP       # raw padded cube 18^3
TQ = S // 2                # 8 wino tiles per row
CCUBE = SP * SP * TQ       # one coeff point-cube: (d18, h18, t8)
NCORES = 8
TSH = S * B // NCORES

_PROGRAM = None


def _build_program():
    import concourse.bacc as bacc
    import concourse.mybir as mybir
    import concourse.tile as tile

    nc = bacc.Bacc("TRN2", target_bir_lowering=False, debug=False,
                   num_devices=NCORES)
    bf16 = mybir.dt.bfloat16
    f32 = mybir.dt.float32
    IDENT = mybir.ActivationFunctionType.Identity
    ADD = mybir.AluOpType.add

    xs_d = nc.dram_tensor("xs", [5, 128, RCUBE], bf16, kind="ExternalInput").ap()
    wl_d = nc.dram_tensor("wl", [128, 36 * 128], bf16, kind="ExternalInput").ap()
    wh_d = nc.dram_tensor("wh", [128, 36 * 128], bf16, kind="ExternalInput").ap()
    bias_d = nc.dram_tensor("bias2", [128, 1], f32, kind="ExternalInput").ap()
    y_d = nc.dram_tensor("y", [TSH, C, S * S * S], bf16,
                         kind="ExternalOutput").ap()

    with tile.TileContext(nc) as tc:
        with (
            tc.tile_pool(name="xr", bufs=2) as rpool,
            tc.tile_pool(name="xc", bufs=3) as cpool,
            tc.tile_pool(name="wp", bufs=1) as wpool,
            tc.tile_pool(name="ev", bufs=2) as epool,
            tc.tile_pool(name="st", bufs=2) as spool,
            tc.tile_pool(name="ps", bufs=8, space="PSUM") as pspool,
        ):
            wlt = wpool.tile([128, 36 * 128], bf16)
            wht = wpool.tile([128, 36 * 128], bf16)
            bias_t = wpool.tile([128, 1], f32)
            # coeff cubes: [128, (c4, d18, h18, t8)] — rotating 3-slot
            # pool: pair u reads cubes u and u+1 only
            vts = []

            wpiece = 9 * 128

            def wdma(q, t, p):
                lo, hi = p * wpiece, (p + 1) * wpiece
                q.dma_start(t[:, lo:hi], (wl_d if t is wlt else wh_d)[:, lo:hi])

            wdma(nc.gpsimd, wlt, 0)
            nc.gpsimd.dma_start(bias_t[:], bias_d)

            # stream raw cubes in d-halves, transform each half right away
            hplanes = 9 * SP * SP  # half a cube (9 d-planes)

            rts = {}

            def load(k):
                rt = rpool.tile([128, RCUBE], bf16, name="rt")
                rts[k] = rt
                for hf in range(2):
                    q0 = nc.sync if k % 2 == 0 else nc.scalar
                    q1 = nc.scalar if k % 2 == 0 else nc.sync
                    lo = hf * hplanes
                    mid = lo + hplanes // 2  # 4.5 planes; element split is fine
                    q0.dma_start(rt[:, lo:mid], xs_d[k][:, lo:mid])
                    q1.dma_start(rt[:, mid:lo + hplanes],
                                 xs_d[k][:, mid:lo + hplanes])

            def transform(k):
                # allocating vt here (not at load) keeps the rotating-slot
                # wait out of the early DVE/Pool queue: vt_k reuses vt_{k-3},
                # whose last readers are pair k-3's L matmuls
                vts.append([cpool.tile([128, CCUBE], bf16, name=f"vt{c}")
                            for c in range(4)])
                rt = rts[k]
                # parity view: w = 2*tt + par, tt in 0..8
                rv = rt.rearrange("p (d h tt par) -> p d h tt par",
                                  d=SP, h=SP, tt=SP // 2, par=2)
                vv = [t_.rearrange("p (d h t) -> p d h t",
                                   d=SP, h=SP, t=TQ) for t_ in vts[k]]

                def wslice(a):  # x[2t+a] for t in 0..7 as a unit-stride AP
                    tt0, par = divmod(a, 2)
                    return (slice(tt0, tt0 + TQ), slice(par, par + 1))

                for hf in range(2):
                    d0, d1 = hf * 9, hf * 9 + 9
                    # c0 = x0 - x2 ; c1 = x1 + x2 ; c2 = x2 - x1 ; c3 = x1 - x3
                    for ci_, (a, b, sub) in enumerate(
                            ((0, 2, True), (1, 2, False), (2, 1, True),
                             (1, 3, True))):
                        eng = nc.vector if (k + ci_) % 2 == 0 else nc.gpsimd
                        sa, pa = wslice(a)
                        sb, pb = wslice(b)
                        in0 = rv[:, d0:d1, :, sa, pa]
                        in1 = rv[:, d0:d1, :, sb, pb]
                        outv = vv[ci_][:, d0:d1, :, :]
                        if sub:
                            eng.tensor_sub(outv, in0, in1)
                        else:
                            eng.tensor_add(outv, in0, in1)

            load(0)
            transform(0)
            for p in (1, 2, 3):
                wdma(nc.gpsimd, wlt, p)
            load(1)
            transform(1)
            for p in range(4):
                wdma(nc.gpsimd, wht, p)
            load(2)
            transform(2)
            load(3)
            load(4)

            def vvs(k, c):
                return vts[k][c].rearrange("p (d h t) -> p d h t",
                                           d=SP, h=SP, t=TQ)

            for u in range(TSH // 2):  # output pair
                if u + 2 < 5 and u + 2 >= 3:
                    transform(u + 2)  # vt3 at pair 1, vt4 at pair 2
                ystage = spool.tile([128, S * S * S], bf16, name="ystage")
                yv2 = ystage.rearrange("p (d h t par) -> p d h t par",
                                       d=S, h=S, t=TQ, par=2)
                a01 = epool.tile([128, 4 * 512], bf16, name="a01")
                a1b = epool.tile([128, 4 * 512], bf16, name="a1b")
                for half in range(2):
                    banks = [pspool.tile([128, 512], f32, name="bank")
                             for _ in range(8)]
                    for blk in range(2):   # L then H
                        wt = wlt if blk == 0 else wht
                        for ph in range(2):
                            c = half * 2 + ph
                            for kd in range(KW):
                                for kh in range(KW):
                                    iw = c * 9 + kd * KW + kh
                                    lhsT = wt[:, iw * 128:(iw + 1) * 128]
                                    for dq in range(4):
                                        rhs = vvs(u + blk, c)[
                                            :,
                                            4 * dq + kd:4 * dq + kd + 4,
                                            kh:kh + S, :]
                                        nc.tensor.matmul(
                                            banks[ph * 4 + dq][:], lhsT, rhs,
                                            start=(blk == 0 and kd == 0
                                                   and kh == 0),
                                            stop=(blk == 1 and kd == 2
                                                  and kh == 2),
                                        )
                    if half == 0:
                        for dq in range(4):
                            sl = slice(dq * 512, (dq + 1) * 512)
                            nc.scalar.activation(a01[:, sl], banks[dq][:],
                                                 IDENT)
                            nc.vector.tensor_scalar_add(a1b[:, sl],
                                                        banks[4 + dq][:],
                                                        bias_t[:])
                    else:
                        qy = nc.sync if u % 2 == 0 else nc.scalar
                        for dq in range(4):
                            sl = slice(dq * 512, (dq + 1) * 512)
                            y0 = yv2[:, 4 * dq:4 * dq + 4, :, :, 0:1]
                            y1 = yv2[:, 4 * dq:4 * dq + 4, :, :, 1:2]
                            t02 = epool.tile([128, 512], bf16, name="t02")
                            t1m3 = epool.tile([128, 512], bf16, name="t1m3")
                            nc.vector.tensor_add(t02[:], a01[:, sl],
                                                 banks[dq][:])
                            nc.gpsimd.tensor_add(y0, t02[:], a1b[:, sl])
                            nc.vector.tensor_sub(t1m3[:], a1b[:, sl],
                                                 banks[4 + dq][:])
                            nc.vector.tensor_sub(y1, t1m3[:], banks[dq][:])
                            cs = slice(dq * 1024, (dq + 1) * 1024)
                            qy.dma_start(y_d[2 * u][:, cs], ystage[0:C, cs])
                            qy.dma_start(y_d[2 * u + 1][:, cs],
                                         ystage[C:128, cs])

    nc.compile()
    return nc


def _host_prep(x, weight, bias):
    import ml_dtypes

    xpad = np.pad(x, ((0, 0), (0, 0), (0, 0), (1, 1), (1, 1), (1, 1)),
                  mode="wrap").astype(ml_dtypes.bfloat16)  # (B,C,S,18,18,18)

    # wino-transformed weights: point c from kw-taps (correlation form)
    #   g0 = w0 ; g1 = (w0+w1+w2)/2 ; g2 = (w0-w1+w2)/2 ; g3 = w2
    w = weight.astype(np.float32)  # (3, co, ci, kd, kh, kw)
    gw = np.stack([
        w[..., 0],
        0.5 * (w[..., 0] + w[..., 1] + w[..., 2]),
        0.5 * (w[..., 0] - w[..., 1] + w[..., 2]),
        w[..., 2],
    ], axis=-1)  # (3, co, ci, kd, kh, c4)

    wl = np.zeros((128, 36, 128), dtype=np.float32)
    wh = np.zeros((128, 36, 128), dtype=np.float32)
    for c in range(4):
        for kd in range(KW):
            for kh in range(KW):
                iw = c * 9 + kd * KW + kh
                for j in range(2):
                    for u in range(2):
                        gl = j - u
                        if 0 <= gl < KW:
                            wl[j * C:(j + 1) * C, iw, u * C:(u + 1) * C] = \
                                gw[gl, :, :, kd, kh, c].T
                        gh = j - u + 2
                        if 0 <= gh < KW:
                            wh[j * C:(j + 1) * C, iw, u * C:(u + 1) * C] = \
                                gw[gh, :, :, kd, kh, c].T
    wl = wl.reshape(128, 36 * 128).astype(ml_dtypes.bfloat16)
    wh = wh.reshape(128, 36 * 128).astype(ml_dtypes.bfloat16)
    bias2 = np.concatenate([bias, bias]).astype(np.float32).reshape(128, 1)

    in_maps = []
    for core in range(NCORES):
        b = core // 2
        t0 = TSH * (core % 2)
        xs = np.empty((5, 128, RCUBE), dtype=ml_dtypes.bfloat16)
        for k in range(5):
            sa = (t0 - 1 + 2 * k) % S
            sb = (t0 + 2 * k) % S
            xs[k, 0:C] = xpad[b, :, sa].reshape(C, RCUBE)
            xs[k, C:128] = xpad[b, :, sb].reshape(C, RCUBE)
        in_maps.append({"xs": xs, "wl": wl, "wh": wh, "bias2": bias2})
    return in_maps


LAST_RESULTS = None


def kernel(x, weight, bias, _trace=False):
    global _PROGRAM, LAST_RESULTS
    from concourse import bass_utils

    x = np.asarray(x, dtype=np.float32)
    weight = np.asarray(weight, dtype=np.float32)
    bias = np.asarray(bias, dtype=np.float32)

    if _PROGRAM is None:
        _PROGRAM = _build_program()
    nc = _PROGRAM

    in_maps = _host_prep(x, weight, bias)
    res = bass_utils.run_bass_kernel_spmd(
        nc, in_maps, core_ids=list(range(NCORES)), trace=_trace
    )
    LAST_RESULTS = res

    out = np.empty((B, C, S, S, S, S), dtype=np.float32)
    for core in range(NCORES):
        b = core // 2
        t0 = TSH * (core % 2)
        y = np.asarray(res.results[core]["y"], dtype=np.float32)
        out[b, :, t0:t0 + TSH] = y.transpose(1, 0, 2).reshape(C, TSH, S, S, S)
    return out


# revision 12
# speedup vs baseline: 1.4982x; 1.0754x over previous
"""Conv4d via 1D-Winograd F(2,3) along w, on 8 TRN2 NeuronCores.

Per output pair (t,t+1) the direct bf16 scheme needs 432 N=512 matmuls; the
w-axis Winograd transform replaces the 3 kw-taps by 4 pointwise products on
half the w-resolution: 288 matmuls -> 246us PE floor instead of 368us.

  input transform  (DVE/Pool, bf16):  per cube, coeffs c0..c3 over (d,h,t8):
      c0 = x[2t]-x[2t+2], c1 = x[2t+1]+x[2t+2],
      c2 = x[2t+2]-x[2t+1], c3 = x[2t+1]-x[2t+3]
  coeff GEMMs (PE): per pair, per point c, the same (j,ci)x(u,co) banded
      L/H time-block structure as the direct kernel, over taps (kd,kh);
      out m_c[(u,co), (d4,h16,t8)] accumulates 18 matmuls in PSUM
  points processed in halves {0,1} then {2,3}; m0/m1 evacuated to SBUF as
      a01 = copy(m0) (Act) and a1b = m1+bias (DVE), then with m2/m3 still
      in PSUM the inverse (one PSUM operand per op — walrus rejects two):
      y[2t]   = (a01 + m2) + a1b           (DVE, then Pool)
      y[2t+1] = (a1b - m3) - m2            (DVE x2)
  all strided window reads/writes use unit-stride parity-split views
  (w = 2*tt+par); stepped (::2) APs also crash walrus codegen
"""
import numpy as np

B, C, S, KW = 4, 64, 16, 3
SP = S + 2
RCUBE = SP * SP * SP       # raw padded cube 18^3
TQ = S // 2                # 8 wino tiles per row
CCUBE = SP * SP * TQ       # one coeff point-cube: (d18, h18, t8)
NCORES = 8
TSH = S * B // NCORES

_PROGRAM = None


def _build_program():
    import concourse.bacc as bacc
    import concourse.mybir as mybir
    import concourse.tile as tile

    nc = bacc.Bacc("TRN2", target_bir_lowering=False, debug=False,
                   num_devices=NCORES)
    bf16 = mybir.dt.bfloat16
    f32 = mybir.dt.float32
    IDENT = mybir.ActivationFunctionType.Identity

    xs_d = nc.dram_tensor("xs", [5, 128, RCUBE], bf16, kind="ExternalInput").ap()
    wl_d = nc.dram_tensor("wl", [128, 36 * 128], bf16, kind="ExternalInput").ap()
    wh_d = nc.dram_tensor("wh", [128, 36 * 128], bf16, kind="ExternalInput").ap()
    bias_d = nc.dram_tensor("bias2", [128, 1], f32, kind="ExternalInput").ap()
    y_d = nc.dram_tensor("y", [TSH, C, S * S * S], bf16,
                         kind="ExternalOutput").ap()

    with tile.TileContext(nc) as tc:
        with (
            tc.tile_pool(name="xr", bufs=2) as rpool,
            tc.tile_pool(name="xc", bufs=3) as cpool,
            tc.tile_pool(name="wp", bufs=1) as wpool,
            tc.tile_pool(name="ev", bufs=2) as epool,
            tc.tile_pool(name="st", bufs=2) as spool,
            tc.tile_pool(name="ps", bufs=8, space="PSUM") as pspool,
        ):
            wlt = wpool.tile([128, 36 * 128], bf16)
            wht = wpool.tile([128, 36 * 128], bf16)
            bias_t = wpool.tile([128, 1], f32)
            # coeff cubes: [128, (c4, d18, h18, t8)] — rotating 3-slot
            # pool: pair u reads cubes u and u+1 only
            vts = []

            wpiece = 9 * 128

            def wdma(q, t, p):
                lo, hi = p * wpiece, (p + 1) * wpiece
                q.dma_start(t[:, lo:hi], (wl_d if t is wlt else wh_d)[:, lo:hi])

            wdma(nc.gpsimd, wlt, 0)
            nc.gpsimd.dma_start(bias_t[:], bias_d)

            # stream raw cubes in d-halves, transform each half right away
            hplanes = 9 * SP * SP  # half a cube (9 d-planes)

            rts = {}

            def load(k):
                rt = rpool.tile([128, RCUBE], bf16, name="rt")
                rts[k] = rt
                for hf in range(2):
                    q0 = nc.sync if k % 2 == 0 else nc.scalar
                    q1 = nc.scalar if k % 2 == 0 else nc.sync
                    lo = hf * hplanes
                    mid = lo + hplanes // 2  # 4.5 planes; element split is fine
                    q0.dma_start(rt[:, lo:mid], xs_d[k][:, lo:mid])
                    q1.dma_start(rt[:, mid:lo + hplanes],
                                 xs_d[k][:, mid:lo + hplanes])

            def transform(k):
                # allocating vt here (not at load) keeps the rotating-slot
                # wait out of the early DVE/Pool queue: vt_k reuses vt_{k-3},
                # whose last readers are pair k-3's L matmuls
                vts.append([cpool.tile([128, CCUBE], bf16, name=f"vt{c}")
                            for c in range(4)])
                rt = rts[k]
                # parity view: w = 2*tt + par, tt in 0..8
                rv = rt.rearrange("p (d h tt par) -> p d h tt par",
                                  d=SP, h=SP, tt=SP // 2, par=2)
                vv = [t_.rearrange("p (d h t) -> p d h t",
                                   d=SP, h=SP, t=TQ) for t_ in vts[k]]

                def wslice(a):  # x[2t+a] for t in 0..7 as a unit-stride AP
                    tt0, par = divmod(a, 2)
                    return (slice(tt0, tt0 + TQ), slice(par, par + 1))

                for hf in range(2):
                    d0, d1 = hf * 9, hf * 9 + 9
                    # c0 = x0 - x2 ; c1 = x1 + x2 ; c2 = x2 - x1 ; c3 = x1 - x3
                    for ci_, (a, b, sub) in enumerate(
                            ((0, 2, True), (1, 2, False), (2, 1, True),
                             (1, 3, True))):
                        eng = nc.vector if (k + ci_) % 2 == 0 else nc.gpsimd
                        sa, pa = wslice(a)
                        sb, pb = wslice(b)
                        in0 = rv[:, d0:d1, :, sa, pa]
                        in1 = rv[:, d0:d1, :, sb, pb]
                        outv = vv[ci_][:, d0:d1, :, :]
                        if sub:
                            eng.tensor_sub(outv, in0, in1)
                        else:
                            eng.tensor_add(outv, in0, in1)

            load(0)
            transform(0)
            for p in (1, 2, 3):
                wdma(nc.gpsimd, wlt, p)
            load(1)
            transform(1)
            for p in range(4):
                wdma(nc.gpsimd, wht, p)
            load(2)
            transform(2)
            load(3)
            load(4)

            def vvs(k, c):
                return vts[k][c].rearrange("p (d h t) -> p d h t",
                                           d=SP, h=SP, t=TQ)

            for u in range(TSH // 2):  # output pair
                if u + 2 < 5 and u + 2 >= 3:
                    transform(u + 2)  # vt3 at pair 1, vt4 at pair 2
                ystage = spool.tile([128, S * S * S], bf16, name="ystage")
                yv2 = ystage.rearrange("p (d h t par) -> p d h t par",
                                       d=S, h=S, t=TQ, par=2)
                a01 = epool.tile([128, 4 * 512], bf16, name="a01")
                a1b = epool.tile([128, 4 * 512], bf16, name="a1b")
                for half in range(2):
                    banks = [pspool.tile([128, 512], f32, name="bank")
                             for _ in range(8)]
                    for blk in range(2):   # L then H
                        wt = wlt if blk == 0 else wht
                        for ph in range(2):
                            c = half * 2 + ph
                            for kd in range(KW):
                                for kh in range(KW):
                                    iw = c * 9 + kd * KW + kh
                                    lhsT = wt[:, iw * 128:(iw + 1) * 128]
                                    for dq in range(4):
                                        rhs = vvs(u + blk, c)[
                                            :,
                                            4 * dq + kd:4 * dq + kd + 4,
                                            kh:kh + S, :]
                                        nc.tensor.matmul(
                                            banks[ph * 4 + dq][:], lhsT, rhs,
                                            start=(blk == 0 and kd == 0
                                                   and kh == 0),
                                            stop=(blk == 1 and kd == 2
                                                  and kh == 2),
                                        )
                    if half == 0:
                        for dq in range(4):
                            sl = slice(dq * 512, (dq + 1) * 512)
                            nc.scalar.activation(a01[:, sl], banks[dq][:],
                                                 IDENT)
                            nc.vector.tensor_scalar_add(a1b[:, sl],
                                                        banks[4 + dq][:],
                                                        bias_t[:])
                    else:
                        qy = nc.sync if u % 2 == 0 else nc.scalar
                        for dq in range(4):
                            sl = slice(dq * 512, (dq + 1) * 512)
                            y0 = yv2[:, 4 * dq:4 * dq + 4, :, :, 0:1]
                            y1 = yv2[:, 4 * dq:4 * dq + 4, :, :, 1:2]
                            t02 = epool.tile([128, 512], bf16, name="t02")
                            t1m3 = epool.tile([128, 512], bf16, name="t1m3")
                            nc.vector.tensor_add(t02[:], a01[:, sl],
                                                 banks[dq][:])
                            nc.gpsimd.tensor_add(y0, t02[:], a1b[:, sl])
                            nc.vector.tensor_sub(t1m3[:], a1b[:, sl],
                                                 banks[4 + dq][:])
                            nc.vector.tensor_sub(y1, t1m3[:], banks[dq][:])
                            cs = slice(dq * 1024, (dq + 1) * 1024)
                            qy.dma_start(y_d[2 * u][:, cs], ystage[0:C, cs])
                            qy.dma_start(y_d[2 * u + 1][:, cs],
                                         ystage[C:128, cs])

    nc.compile()
    return nc


def _host_prep(x, weight, bias):
    import ml_dtypes

    xpad = np.pad(x, ((0, 0), (0, 0), (0, 0), (1, 1), (1, 1), (1, 1)),
                  mode="wrap").astype(ml_dtypes.bfloat16)  # (B,C,S,18,18,18)

    # wino-transformed weights: point c from kw-taps (correlation form)
    #   g0 = w0 ; g1 = (w0+w1+w2)/2 ; g2 = (w0-w1+w2)/2 ; g3 = w2
    w = weight.astype(np.float32)  # (3, co, ci, kd, kh, kw)
    gw = np.stack([
        w[..., 0],
        0.5 * (w[..., 0] + w[..., 1] + w[..., 2]),
        0.5 * (w[..., 0] - w[..., 1] + w[..., 2]),
        w[..., 2],
    ], axis=-1)  # (3, co, ci, kd, kh, c4)

    wl = np.zeros((128, 36, 128), dtype=np.float32)
    wh = np.zeros((128, 36, 128), dtype=np.float32)
    for c in range(4):
        for kd in range(KW):
            for kh in range(KW):
                iw = c * 9 + kd * KW + kh
                for j in range(2):
                    for u in range(2):
                        gl = j - u
                        if 0 <= gl < KW:
                            wl[j * C:(j + 1) * C, iw, u * C:(u + 1) * C] = \
                                gw[gl, :, :, kd, kh, c].T
                        gh = j - u + 2
                        if 0 <= gh < KW:
                            wh[j * C:(j + 1) * C, iw, u * C:(u + 1) * C] = \
                                gw[gh, :, :, kd, kh, c].T
    wl = wl.reshape(128, 36 * 128).astype(ml_dtypes.bfloat16)
    wh = wh.reshape(128, 36 * 128).astype(ml_dtypes.bfloat16)
    bias2 = np.concatenate([bias, bias]).astype(np.float32).reshape(128, 1)

    in_maps = []
    for core in range(NCORES):
        b = core // 2
        t0 = TSH * (core % 2)
        xs = np.empty((5, 128, RCUBE), dtype=ml_dtypes.bfloat16)
        for k in range(5):
            sa = (t0 - 1 + 2 * k) % S
            sb = (t0 + 2 * k) % S
            xs[k, 0:C] = xpad[b, :, sa].reshape(C, RCUBE)
            xs[k, C:128] = xpad[b, :, sb].reshape(C, RCUBE)
        in_maps.append({"xs": xs, "wl": wl, "wh": wh, "bias2": bias2})
    return in_maps


LAST_RESULTS = None


def kernel(x, weight, bias, _trace=False):
    global _PROGRAM, LAST_RESULTS
    from concourse import bass_utils

    x = np.asarray(x, dtype=np.float32)
    weight = np.asarray(weight, dtype=np.float32)
    bias = np.asarray(bias, dtype=np.float32)

    if _PROGRAM is None:
        _PROGRAM = _build_program()
    nc = _PROGRAM

    in_maps = _host_prep(x, weight, bias)
    res = bass_utils.run_bass_kernel_spmd(
        nc, in_maps, core_ids=list(range(NCORES)), trace=_trace
    )
    LAST_RESULTS = res

    out = np.empty((B, C, S, S, S, S), dtype=np.float32)
    for core in range(NCORES):
        b = core // 2
        t0 = TSH * (core % 2)
        y = np.asarray(res.results[core]["y"], dtype=np.float32)
        out[b, :, t0:t0 + TSH] = y.transpose(1, 0, 2).reshape(C, TSH, S, S, S)
    return out
